# revision 93
# baseline (speedup 1.0000x reference)
"""Teacher-forced decoder LSTM on 8 TRN2 NeuronCores.

Problem: B=256, T=32, V=10000, E=H=512 (fp32).
  step s in 0..30: x = embed[caps[:, s]]
                   gates = x@W_ih.T + h@W_hh.T + b     (i,f,g,o)
                   c = sig(f)*c + sig(i)*tanh(g); h = sig(o)*tanh(c)
                   out[s+1] = h@W_lin.T + b_lin
  out[0] = 0.  Output [T, B, V].

Sharding: data-parallel over batch, B_local=32 per core.

Layout: the recurrence runs fully TRANSPOSED (gate/hidden dims on
partitions, batch on the free axis) so each recurrent matmul moves only
32 columns. bf16 weights/activations in the gate path give 1 cyc/row on
the PE at any free size; the logits GEMM stays fp32r off the f32 copy of
h for accuracy.

  phase 1: gather X = embed[tok], PE-transpose, GxT = W_ih@X.T + b as
     bf16 tiles [128, (q)(tok)] per gate type (bias folded in).
  phase 2 (recurrent): per step 4 whole-bank gxT-inject matmuls (start
     the psum group) + 64 W_hh matmuls, all [*, 32/128]-moving bf16; ACT
     sig/tanh straight from PSUM; DVE/Pool cell update; h written
     directly into transposed history (bf16 for the recurrence, f32r for
     the logits GEMM) - no per-step transposes. Logits cols 0:1024 are
     interleaved into the PE stall windows between steps, and the first
     two phase-3 weight super-chunks prefetch on the idle SP DMA queue.
  phase 3: logits cols 1024:10000 as fp32r GEMM streamed per ~1800-col
     super-chunk, stores alternating SP/Pool DMA queues.
"""
import numpy as np

B_FULL, T, V, E, H = 256, 32, 10000, 512, 512
NCORES = 8
BL = B_FULL // NCORES          # 32 batch per core
S = T - 1                      # 31 recurrent steps
M_TOK = S * BL                 # 992 token rows per core
G4 = 4 * H                     # 2048 gate dims
NMT = (M_TOK + 127) // 128     # 8 token m-tiles (last is 96 rows)
V0 = 2048                      # vocab cols done inside phase 2
W3 = 1536                      # phase-3 super-chunk width (3 x 512)
NS3 = 6                        # phase-3 super count (5 x 1536 + 272)

_CACHE = {}


def _build():
    import concourse.bacc as bacc
    import concourse.mybir as mybir
    from concourse.tile import TileContext
    import concourse.bass as bass

    f32 = mybir.dt.float32
    f32r = mybir.dt.float32r
    bf16 = mybir.dt.bfloat16
    i32 = mybir.dt.int32
    SIG = mybir.ActivationFunctionType.Sigmoid
    TANH = mybir.ActivationFunctionType.Tanh
    ADD = mybir.AluOpType.add
    MUL = mybir.AluOpType.mult

    nc = bacc.Bacc()

    emb_d = nc.dram_tensor("emb", [V, E], bf16, kind="ExternalInput")
    wihT_d = nc.dram_tensor("wihT", [E, G4], bf16, kind="ExternalInput")
    whhT_d = nc.dram_tensor("whhT", [H, G4], bf16, kind="ExternalInput")
    biasq_d = nc.dram_tensor("biasq", [128, G4], bf16, kind="ExternalInput")
    wlinT_d = nc.dram_tensor("wlinT", [H, V], f32r, kind="ExternalInput")
    blinb_d = nc.dram_tensor("blinb", [128, V], bf16, kind="ExternalInput")
    tok_d = nc.dram_tensor("tok", [128, NMT], i32, kind="ExternalInput")
    lat_d = nc.dram_tensor("lat", [BL, H], f32r, kind="ExternalInput")
    id128_d = nc.dram_tensor("id128", [128, 128], f32r, kind="ExternalInput")
    id128b_d = nc.dram_tensor("id128b", [128, 128], bf16, kind="ExternalInput")
    out_d = nc.dram_tensor("out", [M_TOK, V], f32, kind="ExternalOutput")

    GATE_ORDER = (2, 0, 1, 3)   # g, i, f, o: start the tanh_g chain early

    with TileContext(nc) as tc:
        with tc.tile_pool(name="const", bufs=1) as cp, \
             tc.tile_pool(name="state", bufs=1) as st:

            # ---------- constants ----------
            tok_sb = cp.tile([128, NMT], i32, tag="tok_sb")
            nc.sync.dma_start(out=tok_sb[:], in_=tok_d[:])
            id128 = cp.tile([128, 128], f32r, tag="id128")
            nc.sync.dma_start(out=id128[:], in_=id128_d[:])
            lat_sb = cp.tile([BL, H], f32r, tag="lat_sb")
            nc.sync.dma_start(out=lat_sb[:], in_=lat_d[:])
            id128b = cp.tile([128, 128], bf16, tag="id128b")
            nc.sync.dma_start(out=id128b[:], in_=id128b_d[:])

            # ---------- state ----------
            hall_f = st.tile([128, 4 * M_TOK], f32r, tag="hall_f")
            hall_b = st.tile([128, 4 * M_TOK], bf16, tag="hall_b")
            hT0 = st.tile([128, 4 * BL], bf16, tag="hT0")   # col = 32k + b
            cT = st.tile([128, 4 * BL], f32, tag="cT")
            nc.vector.memset(cT[:], 0.0)
            actif = st.tile([128, 8 * BL], f32, tag="actif")
            actg = st.tile([128, 4 * BL], f32, tag="actg")
            acto = st.tile([128, 4 * BL], f32, tag="acto")
            t1_sb = st.tile([128, 4 * BL], f32, tag="t1_sb")
            t2_sb = st.tile([128, 4 * BL], f32, tag="t2_sb")
            th_sb = st.tile([128, 4 * BL], f32, tag="th_sb")
            # preload sigmoid/tanh activation tables during setup
            nc.scalar.activation(out=t1_sb[0:1, 0:1], in_=cT[0:1, 0:1], func=SIG)
            nc.scalar.activation(out=t2_sb[0:1, 0:1], in_=cT[0:1, 0:1], func=TANH)

            # phase-3 weight/bias rings (allocated first so they outlive gxp)
            p3w = tc.alloc_tile_pool(name="p3w", bufs=2)
            p3b = tc.alloc_tile_pool(name="p3b", bufs=3)
            sup_bounds = [(V0 + W3 * i, min(V0 + W3 * (i + 1), V)) for i in range(NS3)]

            def load_wl(ns, eng):
                c0, c1 = sup_bounds[ns]
                wl = p3w.tile([128, 4 * W3], f32r, tag="wl", name=f"wl{ns}")
                for k in range(4):
                    eng.dma_start(out=wl[:, W3 * k:W3 * k + (c1 - c0)],
                                  in_=wlinT_d[128 * k:128 * (k + 1), c0:c1])
                return wl

            def load_blin(ns, eng):
                c0, c1 = sup_bounds[ns]
                bl = p3b.tile([128, W3], bf16, tag="bl3", name=f"bl3_{ns}")
                eng.dma_start(out=bl[:, 0:c1 - c0], in_=blinb_d[:, c0:c1])
                return bl

            def wl_pieces(ns, wl):
                # split one wl super-load into small DMAs that drip onto the
                # SP queue between recurrence steps (big transfers would
                # block the interleaved logit stores behind them)
                c0, c1 = sup_bounds[ns]
                ps = []
                for k in range(4):
                    for h0 in range(0, c1 - c0, 512):
                        h1 = min(h0 + 512, c1 - c0)
                        ps.append((wl[:, W3 * k + h0:W3 * k + h1],
                                   wlinT_d[128 * k:128 * (k + 1), c0 + h0:c0 + h1]))
                return ps

            def blin_pieces(ns, bl):
                c0, c1 = sup_bounds[ns]
                ps = []
                for h0 in range(0, c1 - c0, 512):
                    h1 = min(h0 + 512, c1 - c0)
                    ps.append((bl[:, h0:h1], blinb_d[:, c0 + h0:c0 + h1]))
                return ps

            # super0 weights/bias live through phase 3 (mt7 jobs run there)
            wlp = tc.alloc_tile_pool(name="wlp", bufs=1)
            # long-lived recurrence inputs (released before phase 3)
            gxp = tc.alloc_tile_pool(name="gxp", bufs=1)
            whh_sb = gxp.tile([128, 4 * G4], bf16, tag="whh_sb")
            gxT = [gxp.tile([128, 4 * M_TOK], bf16, tag=f"gxT{t}", name=f"gxT{t}")
                   for t in range(4)]
            # super0 (vocab cols 0:V0) weights, interleaved into phase 2
            wl0 = wlp.tile([128, 4 * V0], f32r, tag="wl0")
            blin0 = wlp.tile([128, V0], bf16, tag="blin0")
            # tensors used by deferred phase-1 m-tiles (6,7), whose GEMMs run
            # inside the empty tails of recurrence steps 0..3
            defp = tc.alloc_tile_pool(name="defp", bufs=1)
            wih_sb = defp.tile([128, 4 * G4], bf16, tag="wih_sb")
            biasq = defp.tile([128, G4], bf16, tag="biasq")
            xt_def = [defp.tile([128, 512], bf16, tag=f"xtd{m}", name=f"xtd{m}")
                      for m in (2, 3, 4, 5, 6, 7)]

            spp = tc.alloc_tile_pool(name="spp", bufs=2, space="PSUM")
            sst = tc.alloc_tile_pool(name="sst", bufs=3)

            # ---------- phase 1: gather X, transpose, GxT = W_ih@X.T + b ----------
            with tc.tile_pool(name="p1", bufs=1) as p1, \
                 tc.tile_pool(name="p1ps", bufs=2, space="PSUM") as p1ps, \
                 tc.tile_pool(name="xps", bufs=2, space="PSUM") as xps, \
                 tc.tile_pool(name="tpsum", bufs=1, space="PSUM") as tps:
                # W_ih in 4 k-chunk DMAs so the first GEMM starts early;
                # wl0 after it on the same queue (not needed until step 4)
                for k in range(4):
                    eng = nc.scalar if k % 2 == 0 else nc.sync
                    eng.dma_start(out=wih_sb[:, G4 * k:G4 * (k + 1)],
                                  in_=wihT_d[128 * k:128 * (k + 1), :])
                nc.sync.dma_start(out=biasq[:], in_=biasq_d[:])
                # whh in per-k chunks after wih+biasq: step 0's low-k
                # matmuls can start as soon as their chunk lands
                for k in range(4):
                    nc.sync.dma_start(out=whh_sb[:, G4 * k:G4 * (k + 1)],
                                      in_=whhT_d[128 * k:128 * (k + 1), :])
                nc.sync.dma_start(out=blin0[:], in_=blinb_d[:, 0:V0])
                for k in range(4):
                    nc.scalar.dma_start(out=wl0[:, V0 * k:V0 * (k + 1)],
                                        in_=wlinT_d[128 * k:128 * (k + 1), 0:V0])

                # transpose h0 = latent -> hT0 (one psum tile, one copy)
                pt0 = tps.tile([128, 128], f32r, tag="pt0")
                for k in range(4):
                    nc.tensor.transpose(out=pt0[0:128, BL * k:BL * (k + 1)],
                                        in_=lat_sb[:, 128 * k:128 * (k + 1)],
                                        identity=id128[0:BL, 0:BL])
                nc.vector.tensor_copy(out=hT0[:], in_=pt0[:])

                def gx_gemm(m, xt, pool, tag):
                    rows = min(128, M_TOK - 128 * m)
                    for t in range(4):
                        pg = pool.tile([128, 512], f32, tag=tag, name=f"pg1_{m}_{t}")
                        for q in range(4):
                            for k in range(4):
                                nc.tensor.matmul(
                                    out=pg[:, 128 * q:128 * q + rows],
                                    lhsT=wih_sb[:, G4 * k + 512 * t + 128 * q:
                                                G4 * k + 512 * t + 128 * (q + 1)],
                                    rhs=xt[:, 128 * k:128 * k + rows],
                                    start=(k == 0), stop=(k == 3))
                        # add bias, cast to bf16 (DVE: GPSIMD cannot read PSUM)
                        ai = nc.vector.tensor_tensor(
                            out=gxT[t].rearrange("p (q c) -> p q c", q=4)[:, :, 128 * m:128 * m + rows],
                            in0=pg.rearrange("p (q c) -> p q c", q=4)[:, :, 0:rows],
                            in1=biasq[:, 512 * t:512 * (t + 1)].rearrange(
                                "p (q c) -> p q c", q=4)[:, :, 0:rows],
                            op=ADD)
                        if m >= 2:
                            def_adds.append(ai)

                for m in range(NMT):
                    rows = min(128, M_TOK - 128 * m)
                    x_m = p1.tile([128, E], bf16, tag="x_m", bufs=2, name=f"x_m{m}")
                    nc.gpsimd.indirect_dma_start(
                        out=x_m[0:rows, :], out_offset=None, in_=emb_d[:],
                        in_offset=bass.IndirectOffsetOnAxis(ap=tok_sb[0:rows, m:m + 1], axis=0))
                    xp = xps.tile([128, 512], bf16, tag="xp", name=f"xp{m}")
                    for k in range(4):
                        nc.tensor.transpose(out=xp[0:128, 128 * k:128 * k + rows],
                                            in_=x_m[0:rows, 128 * k:128 * (k + 1)],
                                            identity=id128b[0:rows, 0:rows])
                    xt = (p1.tile([128, 512], bf16, tag="xt", bufs=2, name=f"xt{m}")
                          if m < 2 else xt_def[m - 2])
                    nc.vector.tensor_copy(
                        out=xt.rearrange("p (k c) -> p k c", k=4)[:, :, 0:rows],
                        in_=xp.rearrange("p (k c) -> p k c", k=4)[:, :, 0:rows])
                    # m 2..7: GEMM deferred into the early step tails
                    if m < 2:
                        gx_gemm(m, xt, p1ps, "pg1")

            # deferred m 6,7 GEMMs: deprioritized fillers for steps 0..3
            # (their gxT columns are not consumed until step 24)
            def_adds = []
            with tc.high_priority(offset=-600):
                for dm in (2, 3, 4, 5, 6, 7):
                    gx_gemm(dm, xt_def[dm - 2], spp, "pl0")

            # ---------- phase 2: recurrence with interleaved logits cols 0:V0 ----
            # super jobs: (sup, mt, k) matmuls + fin; emitted into PE stall windows
            s0_jobs = []
            for mt in range(NMT - 1):
                for sup in range(V0 // 512):
                    for k in range(4):
                        s0_jobs.append((sup, mt, k))
            s0_state = {"i": 0, "pl": None, "wl3": 0, "fins": []}

            def super0_next():
                sup, mt, k = s0_jobs[s0_state["i"]]
                s0_state["i"] += 1
                rows = min(128, M_TOK - 128 * mt)
                # deprioritized: fill PE/DVE idle slots, never delay the
                # recurrence chain (the scheduler would otherwise hoist these
                # between critical matmuls)
                with tc.high_priority(offset=-600):
                    if k == 0:
                        s0_state["pl"] = spp.tile([128, 512], f32, tag="pl0",
                                                  name=f"pl0_{sup}_{mt}")
                    pl = s0_state["pl"]
                    # 256-wide quanta (still 1 cyc/row in fp32r): halves the
                    # worst-case overrun past the moment h lands
                    for hh in (0, 256):
                        nc.tensor.matmul(
                            out=pl[0:rows, hh:hh + 256],
                            lhsT=hall_f[:, M_TOK * k + 128 * mt: M_TOK * k + 128 * mt + rows],
                            rhs=wl0[:, V0 * k + 512 * sup + hh: V0 * k + 512 * sup + hh + 256],
                            start=(k == 0 and hh == 0), stop=(k == 3 and hh == 256))
                    if k == 3:
                        stg = sst.tile([128, 512], f32, tag="stg0", name=f"stg0_{sup}_{mt}")
                        # half-width adds; made dependent on the emitting
                        # step's final h-write afterwards, so they can never
                        # slot in front of it on the in-order DVE queue
                        for hh in (0, 256):
                            fi = nc.vector.tensor_tensor(
                                out=stg[0:rows, hh:hh + 256], in0=pl[0:rows, hh:hh + 256],
                                in1=blin0[0:rows, 512 * sup + hh:512 * sup + hh + 256], op=ADD)
                            s0_state["fins"].append(fi)
                        nc.sync.dma_start(
                            out=out_d[128 * mt:128 * mt + rows, 512 * sup:512 * (sup + 1)],
                            in_=stg[0:rows, :])

            hall_b4 = hall_b.rearrange("p (k c) -> p k c", k=4)
            hall_f4 = hall_f.rearrange("p (k c) -> p k c", k=4)
            acto4 = acto.rearrange("p (q b) -> p q b", q=4)
            th4 = th_sb.rearrange("p (q b) -> p q b", q=4)
            # prefetch pieces for phase-3 supers 0,1 (weights + bias), SP queue
            wl3_ring = [p3w.tile([128, 4 * W3], f32r, tag="wl", name="wl0"),
                        p3w.tile([128, 4 * W3], f32r, tag="wl", name="wl1")]
            bl3_ring = [p3b.tile([128, W3], bf16, tag="bl3", name="bl3_0"),
                        p3b.tile([128, W3], bf16, tag="bl3", name="bl3_1")]
            pieces = (wl_pieces(0, wl3_ring[0]) + blin_pieces(0, bl3_ring[0])
                      + wl_pieces(1, wl3_ring[1]) + blin_pieces(1, bl3_ring[1]))
            piece_i = [0]

            with tc.tile_pool(name="rps", bufs=2, space="PSUM") as rps:
                for s in range(S):
                    # i and f share one psum bank (one group, one sig_if ACT)
                    pgif = rps.tile([128, 256], f32, tag="pgif", name=f"pgif_{s}")
                    pgg = rps.tile([128, 128], f32, tag="pgg", name=f"pgg_{s}")
                    pgo = rps.tile([128, 128], f32, tag="pgo", name=f"pgo_{s}")
                    tile_of = {0: pgif, 1: pgif, 2: pgg, 3: pgo}
                    col0 = {0: 0, 1: 128, 2: 0, 3: 0}
                    # whole-bank gxT injects: start each psum group.
                    # Independent of h -> run inside the previous step's tail.
                    for t in GATE_ORDER:
                        nc.tensor.matmul(
                            out=tile_of[t][:, col0[t]:col0[t] + 128],
                            lhsT=id128b[:],
                            rhs=gxT[t].rearrange("p (q c) -> p q c", q=4)[:, :, 32 * s:32 * (s + 1)],
                            start=(t != 1), stop=False)
                    # interleaved logits matmuls: also h(s)-independent tail filler
                    emitted = 0
                    while (emitted < 4 and s0_state["i"] < len(s0_jobs)
                           and s0_jobs[s0_state["i"]][1] < s // 4):
                        super0_next()
                        emitted += 1
                    # drip phase-3 prefetch pieces onto the SP queue
                    if s >= 4:
                        for _ in range(3):
                            if piece_i[0] < len(pieces):
                                dst, src = pieces[piece_i[0]]
                                piece_i[0] += 1
                                nc.sync.dma_start(out=dst, in_=src)
                    # W_hh @ h, order g, i, f, o; k outer so low-k matmuls can
                    # start off the first half of h
                    for t in GATE_ORDER:
                        for k in range(4):
                            for q in range(4):
                                if s == 0:
                                    rh = hT0[:, BL * k:BL * (k + 1)]
                                else:
                                    rh = hall_b[:, M_TOK * k + BL * (s - 1): M_TOK * k + BL * s]
                                nc.tensor.matmul(
                                    out=tile_of[t][:, col0[t] + 32 * q:col0[t] + 32 * (q + 1)],
                                    lhsT=whh_sb[:, G4 * k + 512 * t + 128 * q:
                                                G4 * k + 512 * t + 128 * (q + 1)],
                                    rhs=rh, start=False,
                                    stop=(q == 3 and k == 3 and t != 0))
                        if t == 2:
                            nc.scalar.activation(out=actg[:], in_=pgg[:], func=TANH)
                        elif t == 1:
                            nc.scalar.activation(out=actif[:], in_=pgif[:], func=SIG)
                    # cell update: t1 on Pool, t2 on DVE
                    nc.gpsimd.tensor_tensor(out=t1_sb[:], in0=actif[:, 0:128], in1=actg[:], op=MUL)
                    nc.vector.tensor_tensor(out=t2_sb[:], in0=actif[:, 128:256], in1=cT[:], op=MUL)
                    nc.vector.tensor_tensor(out=cT[:], in0=t1_sb[:], in1=t2_sb[:], op=ADD)
                    # th in halves: th_lo unblocks h_lo (k0-1) a hop earlier
                    nc.scalar.activation(out=th_sb[:, 0:64], in_=cT[:, 0:64], func=TANH)
                    nc.scalar.activation(out=th_sb[:, 64:128], in_=cT[:, 64:128], func=TANH)
                    nc.scalar.activation(out=acto[:], in_=pgo[:], func=SIG)
                    # h (bf16 first, in halves: k0-1 unblocks the next step's
                    # low-k matmuls while k2-3 is still being written)
                    nc.vector.tensor_tensor(out=hall_b4[:, 0:2, BL * s:BL * (s + 1)],
                                            in0=acto4[:, 0:2], in1=th4[:, 0:2], op=MUL)
                    hb_i = nc.vector.tensor_tensor(out=hall_b4[:, 2:4, BL * s:BL * (s + 1)],
                                                   in0=acto4[:, 2:4], in1=th4[:, 2:4], op=MUL)
                    nc.gpsimd.tensor_tensor(out=hall_f4[:, :, BL * s:BL * (s + 1)],
                                            in0=acto4[:], in1=th4[:], op=MUL)
                    import bass_rust as _br
                    _DI = _br.DependencyInfo(sync=True, no_sync=False)
                    for fi in s0_state["fins"]:
                        fi.ins.add_dependency(hb_i.ins.name, _DI)
                    s0_state["fins"] = []
                    for fi in def_adds[:2]:
                        fi.ins.add_dependency(hb_i.ins.name, _DI)
                    del def_adds[:2]
                # drain remaining super jobs and prefetch pieces
                while s0_state["i"] < len(s0_jobs):
                    super0_next()
                while piece_i[0] < len(pieces):
                    dst, src = pieces[piece_i[0]]
                    piece_i[0] += 1
                    nc.sync.dma_start(out=dst, in_=src)

            sst.release()
            spp.release()
            defp.release()
            gxp.release()

            # ---------- phase 3: logits cols V0:10000, fp32r ----------
            with tc.tile_pool(name="p3st", bufs=6) as p3st, \
                 tc.tile_pool(name="p3ps", bufs=2, space="PSUM") as p3ps:
                nst = 0
                # mt7's cols 0:V0 (its h finishes only at step 30) overlap the
                # first wl stream here instead of serializing after the loop
                mt, rows = NMT - 1, M_TOK - 128 * (NMT - 1)
                for sup in range(V0 // 512):
                    pl = p3ps.tile([128, W3], f32, tag="pl", name=f"pl7_{sup}")
                    for k in range(4):
                        nc.tensor.matmul(
                            out=pl[0:rows, 0:512],
                            lhsT=hall_f[:, M_TOK * k + 128 * mt: M_TOK * k + 128 * mt + rows],
                            rhs=wl0[:, V0 * k + 512 * sup: V0 * k + 512 * (sup + 1)],
                            start=(k == 0), stop=(k == 3))
                    stg = p3st.tile([128, W3], f32, tag="stg", name=f"stg7_{sup}")
                    nc.vector.tensor_tensor(out=stg[0:rows, 0:512], in0=pl[0:rows, 0:512],
                                            in1=blin0[0:rows, 512 * sup:512 * (sup + 1)], op=ADD)
                    eng = nc.sync if nst % 2 == 0 else nc.gpsimd
                    nst += 1
                    eng.dma_start(out=out_d[128 * mt:128 * mt + rows, 512 * sup:512 * (sup + 1)],
                                  in_=stg[0:rows, 0:512])
                for ns, (c0, c1) in enumerate(sup_bounds):
                    w_sup = c1 - c0
                    chunks = []
                    off = 0
                    while off < w_sup:
                        chunks.append((off, min(512, w_sup - off)))
                        off += 512
                    wl = wl3_ring[ns]
                    bl = bl3_ring[ns]
                    for m in range(NMT):
                        if m == 1 and ns + 2 < NS3:
                            wl3_ring.append(load_wl(ns + 2, nc.scalar))
                            bl3_ring.append(load_blin(ns + 2, nc.scalar))
                        rows = min(128, M_TOK - 128 * m)
                        pl = p3ps.tile([128, W3], f32, tag="pl")
                        for off, width in chunks:
                            for k in range(4):
                                nc.tensor.matmul(
                                    out=pl[0:rows, off:off + width],
                                    lhsT=hall_f[:, M_TOK * k + 128 * m: M_TOK * k + 128 * m + rows],
                                    rhs=wl[:, W3 * k + off: W3 * k + off + width],
                                    start=(k == 0), stop=(k == 3))
                        stg = p3st.tile([128, W3], f32, tag="stg")
                        nc.vector.tensor_tensor(out=stg[0:rows, 0:w_sup], in0=pl[0:rows, 0:w_sup],
                                                in1=bl[0:rows, 0:w_sup], op=ADD)
                        # rotate store queues (ACT joins once wl loads end)
                        if ns >= NS3 - 2:
                            eng = (nc.sync, nc.gpsimd, nc.scalar)[nst % 3]
                        else:
                            eng = nc.sync if nst % 2 == 0 else nc.gpsimd
                        nst += 1
                        eng.dma_start(out=out_d[128 * m:128 * m + rows, c0:c1],
                                      in_=stg[0:rows, 0:w_sup])

            wlp.release()
            p3b.release()
            p3w.release()

    nc.compile()
    return nc


def _prep_host(caps, latent, embed, W_ih, W_hh, b_ih, b_hh, W_lin, b_lin):
    import ml_dtypes
    bf = ml_dtypes.bfloat16
    caps = np.asarray(caps).astype(np.int32)
    latent = np.asarray(latent, dtype=np.float32)
    embed = np.ascontiguousarray(np.asarray(embed, dtype=np.float32).astype(bf))
    wihT = np.ascontiguousarray(np.asarray(W_ih, dtype=np.float32).T.astype(bf))  # [E, 4H]
    whhT = np.ascontiguousarray(np.asarray(W_hh, dtype=np.float32).T.astype(bf))  # [H, 4H]
    bias = (np.asarray(b_ih, dtype=np.float32) + np.asarray(b_hh, dtype=np.float32))
    # biasq[p, c] = bias[(c//128)*128 + p]
    blk = bias.reshape(G4 // 128, 128)            # [16, 128]
    biasq = np.ascontiguousarray(
        np.broadcast_to(blk.T[:, :, None], (128, G4 // 128, 128))
        .reshape(128, G4).astype(bf))
    wlinT = np.ascontiguousarray(np.asarray(W_lin, dtype=np.float32).T)   # [H, V]
    blinb = np.ascontiguousarray(np.broadcast_to(
        np.asarray(b_lin, dtype=np.float32)[None, :], (128, V)).astype(bf))
    id128 = np.eye(128, dtype=np.float32)
    id128b = np.eye(128).astype(bf)

    in_maps = []
    for c in range(NCORES):
        caps_sh = caps[c * BL:(c + 1) * BL]                   # [32, 32]
        tok_flat = caps_sh[:, :S].T.reshape(M_TOK)            # t-major [992]
        tok_pad = np.zeros(NMT * 128, dtype=np.int32)
        tok_pad[:M_TOK] = tok_flat
        tok = np.ascontiguousarray(tok_pad.reshape(NMT, 128).T)  # [128, NMT]
        in_maps.append(dict(
            emb=embed, wihT=wihT, whhT=whhT, biasq=biasq, wlinT=wlinT,
            blinb=blinb, tok=tok, lat=np.ascontiguousarray(latent[c * BL:(c + 1) * BL]),
            id128=id128, id128b=id128b,
        ))
    return in_maps


def kernel(caps, latent, embed, W_ih, W_hh, b_ih, b_hh, W_lin, b_lin):
    from concourse.bass_utils import run_bass_kernel_spmd

    if "nc" not in _CACHE:
        _CACHE["nc"] = _build()
    nc = _CACHE["nc"]

    in_maps = _prep_host(caps, latent, embed, W_ih, W_hh, b_ih, b_hh, W_lin, b_lin)
    res = run_bass_kernel_spmd(nc, in_maps, core_ids=list(range(NCORES)))
    out = np.zeros((T, B_FULL, V), dtype=np.float32)
    for c in range(NCORES):
        shard = res.results[c]["out"].reshape(S, BL, V)
        out[1:, c * BL:(c + 1) * BL, :] = shard
    return out
